# revision 1
# baseline (speedup 1.0000x reference)
"""GQA kernel for trn2, 8 NeuronCores.

Sharding: DP over batch (2) x TP over heads (4 groups):
core c -> batch c//4, head-group g=c%4 (q-heads 8g..8g+7, kv-heads 2g,2g+1,
wq/wk/wv column-slices, wo row-slice). Each core computes a partial [T, D]
output for its batch; host sums the 4 partials per batch.

On-core: x^T (host pre-transposed) streams in; Q^T/K^T/V^T computed via
matmul with weights stationary (f32r, full PE rate); attention computed in
S^T layout (k on partitions) so no transposes are needed anywhere except
V (tiny 128x128 TensorE transposes); softmax normalization is folded as
1/rowsum multiply on the attention output; final projection contracts the
per-core 512 head-cols against the wo row-slice.
"""
import sys
sys.path.insert(0, '/opt/trn_rl_repo')
import numpy as np

B, T, D = 2, 2048, 2048
HEADS_PER_CORE = 8      # q heads per core
KV_PER_CORE = 2
DH = 64
SCALE = 0.125           # 1/sqrt(64)
NQB = 4                 # q blocks of 512
NTQ = 4                 # T quarters for projection streaming
KIN = 16                # contraction tiles over D
NCORES = 8

_nc_cache = {}


def _build():
    if "nc" in _nc_cache:
        return _nc_cache["nc"]
    import concourse.bass as bass
    from concourse import bacc, mybir
    global mybir_mod
    mybir_mod = mybir
    import concourse.tile as tile
    from concourse.masks import make_identity

    f32 = mybir.dt.float32
    f32r = mybir.dt.float32r
    AF = mybir.ActivationFunctionType

    nc = bacc.Bacc()
    xt = nc.declare_dram_parameter("xt", [D, T], f32r, isOutput=False)
    wq = nc.declare_dram_parameter("wq", [D, 512], f32r, isOutput=False)
    wk = nc.declare_dram_parameter("wk", [D, 128], f32r, isOutput=False)
    wv = nc.declare_dram_parameter("wv", [D, 128], f32r, isOutput=False)
    wo = nc.declare_dram_parameter("wo", [512, D], f32r, isOutput=False)
    vconst = nc.declare_dram_parameter("vconst", [128, KV_PER_CORE, 17, 128], f32r,
                                       isOutput=False)
    out = nc.declare_dram_parameter("out", [T, D], f32, isOutput=True)

    wq_r = wq.rearrange("(kin p) m -> kin p m", p=128)
    wk_r = wk.rearrange("(kin p) m -> kin p m", p=128)
    wv_r = wv.rearrange("(kin p) m -> kin p m", p=128)
    wo_r = wo.rearrange("(c p) n -> c p n", p=128)
    xt_r = xt.rearrange("(kin p) t -> kin p t", p=128)

    with tile.TileContext(nc) as tc:
        with tc.tile_pool(name="wbig", bufs=1) as wbig, \
             tc.tile_pool(name="wsmall", bufs=1) as wsmall, \
             tc.tile_pool(name="persist", bufs=1) as persist, \
             tc.tile_pool(name="xtp", bufs=6) as xtp, \
             tc.tile_pool(name="exps", bufs=4) as exps, \
             tc.tile_pool(name="small", bufs=4) as small, \
             tc.tile_pool(name="yout", bufs=3) as yout:

            # ---- resident weights ----
            wq_sb = wbig.tile([128, KIN, 512], f32r, tag="wbig")
            wk_sb = wsmall.tile([128, KIN, 128], f32r, tag="wk")
            wv_sb = wsmall.tile([128, KIN, 128], f32r, tag="wv")
            for kin in range(KIN):
                nc.sync.dma_start(out=wq_sb[:, kin, :], in_=wq_r[kin])
                nc.sync.dma_start(out=wk_sb[:, kin, :], in_=wk_r[kin])
                nc.sync.dma_start(out=wv_sb[:, kin, :], in_=wv_r[kin])

            ident = persist.tile([128, 128], f32)
            make_identity(nc, ident)

            # ---- persistent activations ----
            # QT: 4 chunks of [128, T] (q head-cols on partitions)
            qt_sb = persist.tile([128, 4, T], f32r)
            # KT: [128, T]; rows 0-63 = kv0 K^T, 64-127 = kv1 K^T
            kt_sb = persist.tile([128, T], f32r)
            # V natural layout + ones col: per kv head, 16 tiles.
            # kv0: cols 0-63 = V, col 64 = ones  -> O at partitions 0-63, sums at 64
            # kv1: col 0 = ones, cols 64-127 = V -> sums at partition 0, O at 64-127
            v_sb = persist.tile([128, KV_PER_CORE, 17, 128], f32r)
            # attention out (pre-wo), lhsT layout: 4 chunks [128, T]
            ot_sb = persist.tile([128, 4, T], f32r)

            for kv in range(KV_PER_CORE):
                nc.sync.dma_start(out=v_sb[:, kv], in_=vconst[:, kv])

            # ---- phase B: projections (stream x^T in T-quarters) ----
            pb = tc.tile_pool(name="pps", bufs=6, space="PSUM")
            pps = pb.__enter__()
            tb = tc.tile_pool(name="tps", bufs=2, space="PSUM")
            tps = tb.__enter__()
            for tq in range(NTQ):
                ts_ = slice(tq * 512, (tq + 1) * 512)
                qps = []
                for mc in range(4):
                    qp_t = pps.tile([128, 512], f32, tag="ps")
                    qps.append(qp_t)
                kps = pps.tile([128, 512], f32, tag="ps")
                vps = pps.tile([128, 512], f32, tag="ps")
                for kin in range(KIN):
                    xtile = xtp.tile([128, 512], f32r, tag="xt")
                    nc.sync.dma_start(out=xtile, in_=xt_r[kin][:, ts_])
                    st, sp = (kin == 0), (kin == KIN - 1)
                    for mc in range(4):
                        nc.tensor.matmul(qps[mc], wq_sb[:, kin, mc * 128:(mc + 1) * 128],
                                         xtile, start=st, stop=sp)
                    nc.tensor.matmul(kps, wk_sb[:, kin, :], xtile, start=st, stop=sp)
                    nc.tensor.matmul(vps, wv_sb[:, kin, :], xtile, start=st, stop=sp)
                for mc in range(4):
                    nc.vector.tensor_copy(out=qt_sb[:, mc, ts_], in_=qps[mc])
                nc.vector.tensor_copy(out=kt_sb[:, ts_], in_=kps)
                # V^T chunk -> transpose to natural V tiles
                vt_sb = small.tile([128, 512], f32, tag="vt")
                nc.vector.tensor_copy(out=vt_sb, in_=vps)
                for st4 in range(4):
                    tt = tq * 4 + st4
                    trp = tps.tile([128, 128], f32, tag="tp")
                    nc.tensor.transpose(trp, vt_sb[:, st4 * 128:(st4 + 1) * 128], ident)
                    nc.vector.tensor_copy(out=v_sb[:, 0, tt, 0:64], in_=trp[:, 0:64])
                    nc.vector.tensor_copy(out=v_sb[:, 1, tt, 64:128], in_=trp[:, 64:128])

            tb.__exit__(None, None, None)
            pb.__exit__(None, None, None)

            # ---- phase C+D fused: attention (qb outer) + output proj per q-block ----
            sb_ = tc.tile_pool(name="spp", bufs=5, space="PSUM")
            spp = sb_.__enter__()
            ob_ = tc.tile_pool(name="opp", bufs=3, space="PSUM")
            opp = ob_.__enter__()
            # wo shares the wbig slot with wq (wq released after projections);
            # loading here lets the DMA overlap the start of attention
            wo_sb = wbig.tile([128, 4, T], f32r, tag="wbig")
            for c in range(4):
                nc.sync.dma_start(out=wo_sb[:, c, :], in_=wo_r[c])
            for qb in range(NQB):
                qs = slice(qb * 512, (qb + 1) * 512)
                nkt = 4 * (qb + 1)
                for h in range(HEADS_PER_CORE):
                    kv = h // 4
                    mc = h % 4          # host packs head h with head h+4 in chunk h%4
                    row0 = 64 * kv      # h<4 at partitions 0-63, h>=4 at 64-127
                    q_rows = slice(row0, row0 + 64)
                    k_rows = slice(row0, row0 + 64)
                    o_ps = opp.tile([128, 512], f32, tag="op")
                    prev = None
                    for kt in range(nkt):
                        s_ps = spp.tile([128, 512], f32, tag="sp")
                        nc.tensor.matmul(s_ps,
                                         kt_sb[k_rows, kt * 128:(kt + 1) * 128],
                                         qt_sb[q_rows, mc, qs],
                                         start=True, stop=True)
                        e_sb = exps.tile([128, 512], f32r, tag="ex")
                        nc.scalar.activation(out=e_sb, in_=s_ps, func=AF.Exp, scale=SCALE)
                        if kt >= 4 * qb:
                            nc.gpsimd.affine_select(
                                out=e_sb, in_=e_sb,
                                pattern=[[1, 512]],
                                compare_op=mybir.AluOpType.is_ge,
                                fill=0.0,
                                base=-128 * (kt - 4 * qb),
                                channel_multiplier=-1)
                        # software-pipeline the PV matmul one step behind
                        if prev is not None:
                            pkt, pe = prev
                            vl = v_sb[:, 0, pkt, 0:65] if kv == 0 else v_sb[:, 1, pkt, :]
                            nc.tensor.matmul(o_ps[0:65, :] if kv == 0 else o_ps,
                                             vl, pe, start=(pkt == 0), stop=False)
                        prev = (kt, e_sb)
                    pkt, pe = prev
                    vl = v_sb[:, 0, pkt, 0:65] if kv == 0 else v_sb[:, 1, pkt, :]
                    nc.tensor.matmul(o_ps[0:65, :] if kv == 0 else o_ps,
                                     vl, pe, start=(pkt == 0), stop=True)
                    # normalize: O rows / sums row (layout depends on kv)
                    srow = slice(64, 65) if kv == 0 else slice(0, 1)
                    orow = slice(0, 64) if kv == 0 else slice(64, 128)
                    r_sb = small.tile([128, 512], f32r, tag="r")
                    with nc.allow_low_precision(reason="f32r reciprocal for matmul rhs"):
                        nc.vector.reciprocal(out=r_sb[srow, :], in_=o_ps[srow, :])
                    # broadcast r across partitions: ones[1,128].T @ r[1,512]
                    ob0 = 64 - row0   # partition where the sums row lives
                    ones_row = v_sb[ob0:ob0 + 1, 0, 16, 0:128]
                    rb_ps = spp.tile([128, 512], f32, tag="sp")
                    nc.tensor.matmul(rb_ps, ones_row, r_sb[srow, :],
                                     start=True, stop=True)
                    rb_sb = small.tile([128, 512], f32, tag="rb")
                    nc.vector.tensor_copy(out=rb_sb[orow, :], in_=rb_ps[orow, :])
                    nc.vector.tensor_tensor(
                        out=ot_sb[q_rows, mc, qs],
                        in0=o_ps[orow, :], in1=rb_sb[orow, :],
                        op=mybir.AluOpType.mult)
                # output projection for this q-block (overlaps next qb's attention)
                for tt in range(4 * qb, 4 * qb + 4):
                    tsl = slice(tt * 128, (tt + 1) * 128)
                    for nb in range(4):
                        nsl = slice(nb * 512, (nb + 1) * 512)
                        y_ps = opp.tile([128, 512], f32, tag="op")
                        for c in range(4):
                            nc.tensor.matmul(y_ps, ot_sb[:, c, tsl], wo_sb[:, c, nsl],
                                             start=(c == 0), stop=(c == 3))
                        y_sb = yout.tile([128, 512], f32, tag="y")
                        if (tt * 4 + nb) % 2 == 0:
                            nc.vector.tensor_copy(out=y_sb, in_=y_ps)
                        else:
                            nc.scalar.activation(out=y_sb, in_=y_ps, func=AF.Copy)
                        nc.sync.dma_start(out=out[tsl, nsl], in_=y_sb)
            ob_.__exit__(None, None, None)
            sb_.__exit__(None, None, None)

    nc.finalize()
    _nc_cache["nc"] = nc
    return nc


_HEAD_ORDER = [0, 4, 1, 5, 2, 6, 3, 7]

_VCONST = np.zeros((128, KV_PER_CORE, 17, 128), dtype=np.float32)
_VCONST[:, 0, :16, 64] = 1.0
_VCONST[:, 1, :16, 0] = 1.0
# slot 16 = all-ones rows for the softmax-sum broadcast matmul
_VCONST[:, :, 16, :] = 1.0


def _perm_wq(wq, g):
    cols = wq[:, 512 * g:512 * (g + 1)].reshape(D, 8, DH)
    return np.ascontiguousarray(cols[:, _HEAD_ORDER].reshape(D, 512))


def _perm_wo(wo, g):
    rows = wo[512 * g:512 * (g + 1), :].reshape(8, DH, D)
    return np.ascontiguousarray(rows[_HEAD_ORDER].reshape(512, D))


def kernel(x, wq, wk, wv, wo, attention_mask=None, **_ignored):
    from concourse.bass_utils import run_bass_kernel_spmd

    x = np.asarray(x, dtype=np.float32)
    wq = np.asarray(wq, dtype=np.float32)
    wk = np.asarray(wk, dtype=np.float32)
    wv = np.asarray(wv, dtype=np.float32)
    wo = np.asarray(wo, dtype=np.float32)

    nc = _build()
    in_maps = []
    for c in range(NCORES):
        bi, g = c // 4, c % 4
        in_maps.append({
            "vconst": _VCONST,
            "xt": np.ascontiguousarray(x[bi].T),
            "wq": _perm_wq(wq, g),
            "wk": np.ascontiguousarray(wk[:, 128 * g:128 * (g + 1)]),
            "wv": np.ascontiguousarray(wv[:, 128 * g:128 * (g + 1)]),
            "wo": _perm_wo(wo, g),
        })
    res = run_bass_kernel_spmd(nc, in_maps, list(range(NCORES)))
    y = np.zeros((B, T, D), dtype=np.float32)
    for c in range(NCORES):
        y[c // 4] += res.results[c]["out"]
    return y



# revision 23
# speedup vs baseline: 9605.7048x; 9605.7048x over previous
"""GQA kernel for trn2, 8 NeuronCores.

Sharding: DP over batch (2) x TP over heads (4 groups):
core c -> batch c//4, head-group g=c%4 (q-heads 8g..8g+7, kv-heads 2g,2g+1,
wq/wk/wv column-slices, wo row-slice). Each core computes a partial [T, D]
output for its batch; host sums the 4 partials per batch.

On-core: x^T (host pre-transposed, fp16) is preloaded with 16 large
contiguous DMAs; Q^T/K^T/V^T computed via matmul with weights stationary;
attention computed in S^T layout (k on partitions). The causal mask is
preloaded into PSUM by a PE matmul (identity x mask-tile) so it never sits
between exp and PV on the critical path. Two heads run in lockstep with
kt-tiles processed in groups of 4 so the PE gets long uninterrupted runs
while ACT exps the other head's group. Softmax normalization is folded as
1/rowsum multiply on the attention output; output rows are written with
contiguous full-row DMAs. All HBM traffic is fp16; accumulation is fp32.
"""
import sys
sys.path.insert(0, '/opt/trn_rl_repo')
import numpy as np

B, T, D = 2, 2048, 2048
HEADS_PER_CORE = 8      # q heads per core
KV_PER_CORE = 2
DH = 64
SCALE = 0.125           # 1/sqrt(64)
NQB = 4                 # q blocks of 512
NTQ = 4                 # T quarters for projection streaming
KIN = 16                # contraction tiles over D
NCORES = 8

_nc_cache = {}
ABLATION = "full"   # "B" = projections only, "BA" = + attention, "full" = everything


def _build(reps=1):
    key = f"nc{reps}-{ABLATION}"
    if key in _nc_cache:
        return _nc_cache[key]
    import concourse.bass as bass
    from concourse import bacc, mybir
    global mybir_mod
    mybir_mod = mybir
    import concourse.tile as tile
    from concourse.masks import make_identity

    f32 = mybir.dt.float32
    f16 = mybir.dt.float16
    AF = mybir.ActivationFunctionType

    nc = bacc.Bacc()
    xt = nc.declare_dram_parameter("xt", [D, T], f16, isOutput=False)
    wq = nc.declare_dram_parameter("wq", [D, 512], f16, isOutput=False)
    wk = nc.declare_dram_parameter("wk", [D, 128], f16, isOutput=False)
    wv = nc.declare_dram_parameter("wv", [D, 128], f16, isOutput=False)
    wo = nc.declare_dram_parameter("wo", [512, D], f16, isOutput=False)
    out = nc.declare_dram_parameter("out", [T, D], f16, isOutput=True)

    wq_r = wq.rearrange("(kin p) m -> kin p m", p=128)
    wk_r = wk.rearrange("(kin p) m -> kin p m", p=128)
    wv_r = wv.rearrange("(kin p) m -> kin p m", p=128)
    wo_r = wo.rearrange("(c p) n -> c p n", p=128)
    xt_r = xt.rearrange("(kin p) t -> kin p t", p=128)

    with tile.TileContext(nc) as tc:
        with tc.tile_pool(name="wbig", bufs=1) as wbig, \
             tc.tile_pool(name="wsmall", bufs=1) as wsmall, \
             tc.tile_pool(name="persist", bufs=1) as persist, \
             tc.tile_pool(name="exps", bufs=16) as exps, \
             tc.tile_pool(name="small", bufs=4) as small, \
             tc.tile_pool(name="yout", bufs=3) as yout:

            for _rep in range(reps):
                # ---- resident weights + full x^T (all contiguous DMAs) ----
                wq_sb = wbig.tile([128, KIN, 512], f16, tag="wbig")
                wk_sb = wsmall.tile([128, KIN, 128], f16, tag="wk")
                wv_sb = wsmall.tile([128, KIN, 128], f16, tag="wv")
                xt_sb = persist.tile([128, KIN, T], f16)
                for kin in range(KIN):
                    nc.sync.dma_start(out=wq_sb[:, kin, :], in_=wq_r[kin])
                    nc.sync.dma_start(out=wk_sb[:, kin, :], in_=wk_r[kin])
                    nc.sync.dma_start(out=wv_sb[:, kin, :], in_=wv_r[kin])
                    nc.sync.dma_start(out=xt_sb[:, kin, :], in_=xt_r[kin])

                ident = persist.tile([128, 128], f16)
                make_identity(nc, ident)

                # ---- persistent activations ----
                # QT: 4 chunks of [128, T] (q head-cols on partitions)
                qt_sb = persist.tile([128, 4, T], f16)
                # KT: [128, T]; rows 0-63 = kv0 K^T, 64-127 = kv1 K^T
                kt_sb = persist.tile([128, T], f16)
                # V natural layout + ones col: per kv head, 16 tiles.
                # kv0: cols 0-63 = V, col 64 = ones  -> O at partitions 0-63, sums at 64
                # kv1: col 0 = ones, cols 64-127 = V -> sums at partition 0, O at 64-127
                v_sb = persist.tile([128, KV_PER_CORE, 17, 128], f16)
                # attention out (pre-wo), lhsT layout: 4 chunks [128, T]
                ot_sb = persist.tile([128, 4, T], f16)

                nc.gpsimd.memset(v_sb[:, 0, 0:16, 64:128], 0.0)
                nc.gpsimd.memset(v_sb[:, 1, 0:16, 0:64], 0.0)
                nc.gpsimd.memset(v_sb[:, 0, 0:17, 64:65], 1.0)
                nc.gpsimd.memset(v_sb[:, 1, 0:17, 0:1], 1.0)
                nc.gpsimd.memset(v_sb[:, 0, 16, 0:128], 1.0)
                nc.gpsimd.memset(v_sb[:, 1, 16, 0:128], 1.0)

                # ---- phase B: projections ----
                pb = tc.tile_pool(name="pps", bufs=6, space="PSUM")
                pps = pb.__enter__()
                tb = tc.tile_pool(name="tps", bufs=2, space="PSUM")
                tps = tb.__enter__()
                for tq in range(NTQ):
                    ts_ = slice(tq * 512, (tq + 1) * 512)
                    qps = []
                    for mc in range(4):
                        qp_t = pps.tile([128, 512], f32, tag="ps")
                        qps.append(qp_t)
                    kps = pps.tile([128, 512], f32, tag="ps")
                    vps = pps.tile([128, 512], f32, tag="ps")
                    # bank-major: all 16 contraction steps of one PSUM tile
                    # run consecutively (much faster than interleaving banks)
                    for mc in range(4):
                        for kin in range(KIN):
                            nc.tensor.matmul(qps[mc], wq_sb[:, kin, mc * 128:(mc + 1) * 128],
                                             xt_sb[:, kin, ts_],
                                             start=(kin == 0), stop=(kin == KIN - 1))
                    for kin in range(KIN):
                        nc.tensor.matmul(kps, wk_sb[:, kin, :], xt_sb[:, kin, ts_],
                                         start=(kin == 0), stop=(kin == KIN - 1))
                    for kin in range(KIN):
                        nc.tensor.matmul(vps, wv_sb[:, kin, :], xt_sb[:, kin, ts_],
                                         start=(kin == 0), stop=(kin == KIN - 1))
                    for mc in range(4):
                        nc.vector.tensor_copy(out=qt_sb[:, mc, ts_], in_=qps[mc])
                    nc.vector.tensor_copy(out=kt_sb[:, ts_], in_=kps)
                    # V^T chunk -> transpose to natural V tiles
                    vt_sb = small.tile([128, 512], f16, tag="vt")
                    nc.vector.tensor_copy(out=vt_sb, in_=vps)
                    for st4 in range(4):
                        tt = tq * 4 + st4
                        trp = tps.tile([128, 128], f16, tag="tp")
                        nc.tensor.transpose(trp, vt_sb[:, st4 * 128:(st4 + 1) * 128], ident)
                        nc.vector.tensor_copy(out=v_sb[:, 0, tt, 0:64], in_=trp[:, 0:64])
                        nc.vector.tensor_copy(out=v_sb[:, 1, tt, 64:128], in_=trp[:, 64:128])

                tb.__exit__(None, None, None)
                pb.__exit__(None, None, None)

                if ABLATION == "B":
                    continue
                # ---- phase C+D fused: attention (qb outer) + output proj per q-block ----
                sb_ = tc.tile_pool(name="spp", bufs=4, space="PSUM")
                spp = sb_.__enter__()
                ob_ = tc.tile_pool(name="opp", bufs=2, space="PSUM")
                opp = ob_.__enter__()
                yb_ = tc.tile_pool(name="ypp", bufs=2, space="PSUM")
                ypp = yb_.__enter__()
                # wo shares the wbig slot with wq (wq released after projections);
                # loading here lets the DMA overlap the start of attention
                wo_sb = wbig.tile([128, 4, T], f16, tag="wbig")
                if ABLATION == "full":
                    for c in range(4):
                        nc.sync.dma_start(out=wo_sb[:, c, :], in_=wo_r[c])
                for qb in range(NQB):
                    qs = slice(qb * 512, (qb + 1) * 512)
                    nkt = 4 * (qb + 1)

                    def s_step(h, kt, state):
                        kv, mc = h // 4, h % 4
                        row0 = 64 * kv
                        s_ps = spp.tile([128, 512], f32, tag="sp")
                        nc.tensor.matmul(s_ps,
                                         kt_sb[row0:row0 + 64, kt * 128:(kt + 1) * 128],
                                         qt_sb[row0:row0 + 64, mc, qs],
                                         start=True, stop=True)
                        e_sb = exps.tile([128, 512], f16, tag="ex")
                        nc.scalar.activation(out=e_sb, in_=s_ps, func=AF.Exp, scale=SCALE)
                        if kt >= 4 * qb:
                            # causal mask on the fp16 e tile in SBUF (Pool is
                            # otherwise idle; the group-lagged PV gives this
                            # plenty of slack before the e tile is consumed)
                            nc.gpsimd.affine_select(
                                out=e_sb, in_=e_sb,
                                pattern=[[1, 512]],
                                compare_op=mybir.AluOpType.is_ge,
                                fill=0.0,
                                base=-128 * (kt - 4 * qb),
                                channel_multiplier=-1)
                        state["pend"].append((kt, e_sb))

                    def pv_step(h, state, final=False):
                        kv = h // 4
                        pkt, pe = state["pend"].pop(0)
                        vl = v_sb[:, 0, pkt, 0:65] if kv == 0 else v_sb[:, 1, pkt, :]
                        o_ps = state["o_ps"]
                        nc.tensor.matmul(o_ps[0:65, :] if kv == 0 else o_ps,
                                         vl, pe, start=(pkt == 0),
                                         stop=final and not state["pend"])

                    def tail(h, state):
                        # normalize: O rows / sums row (layout depends on kv)
                        kv, mc = h // 4, h % 4
                        row0 = 64 * kv
                        o_ps = state["o_ps"]
                        srow = slice(64, 65) if kv == 0 else slice(0, 1)
                        orow = slice(0, 64) if kv == 0 else slice(64, 128)
                        r_sb = small.tile([128, 512], f16, tag="r")
                        with nc.allow_low_precision(reason="fp16 reciprocal for matmul rhs"):
                            nc.vector.reciprocal(out=r_sb[srow, :], in_=o_ps[srow, :])
                        # broadcast r across partitions: ones[1,128].T @ r[1,512]
                        ob0 = 64 - row0   # partition where the sums row lives
                        ones_row = v_sb[ob0:ob0 + 1, 0, 16, 0:128]
                        rb_ps = spp.tile([128, 512], f32, tag="sp")
                        nc.tensor.matmul(rb_ps, ones_row, r_sb[srow, :],
                                         start=True, stop=True)
                        rb_sb = small.tile([128, 512], f32, tag="rb")
                        nc.vector.tensor_copy(out=rb_sb[orow, :], in_=rb_ps[orow, :])
                        nc.vector.tensor_tensor(
                            out=ot_sb[row0:row0 + 64, mc, qs],
                            in0=o_ps[orow, :], in1=rb_sb[orow, :],
                            op=mybir_mod.AluOpType.mult)

                    # two heads in lockstep, kt tiles in groups of 4: the PE
                    # runs a head's 4 S matmuls back to back while ACT exps
                    # the sibling's group; PV lags one full group.
                    for pair in range(4):
                        heads = (2 * pair, 2 * pair + 1)
                        st = {h: {"o_ps": opp.tile([128, 512], f32, tag="op",
                                                   name=f"o_ps_h{h}"),
                                  "pend": []} for h in heads}
                        for g in range(qb + 1):
                            for h in heads:
                                for kt in range(4 * g, 4 * g + 4):
                                    s_step(h, kt, st[h])
                            if g > 0:
                                for h in heads:
                                    for _ in range(4):
                                        pv_step(h, st[h])
                        for h in heads:
                            while st[h]["pend"]:
                                pv_step(h, st[h], final=True)
                            tail(h, st[h])
                    if ABLATION == "BA":
                        continue
                    # output projection for this q-block: full output rows,
                    # one contiguous row-block DMA per 128 rows. Two n-blocks
                    # run per c so consecutive matmuls share the stationary
                    # lhsT (cheaper weight loads).
                    for tt in range(4 * qb, 4 * qb + 4):
                        tsl = slice(tt * 128, (tt + 1) * 128)
                        y_row = yout.tile([128, T], f16, tag="y")
                        for np2 in range(2):
                            yps2 = [ypp.tile([128, 512], f32, tag="yp",
                                             name=f"y_ps{nb}") for nb in range(2)]
                            for c in range(4):
                                for nb in range(2):
                                    nsl = slice((2 * np2 + nb) * 512,
                                                (2 * np2 + nb + 1) * 512)
                                    nc.tensor.matmul(yps2[nb], ot_sb[:, c, tsl],
                                                     wo_sb[:, c, nsl],
                                                     start=(c == 0), stop=(c == 3))
                            for nb in range(2):
                                nsl = slice((2 * np2 + nb) * 512,
                                            (2 * np2 + nb + 1) * 512)
                                nc.vector.tensor_copy(out=y_row[:, nsl], in_=yps2[nb])
                        nc.sync.dma_start(out=out[tsl, :], in_=y_row)
                yb_.__exit__(None, None, None)
                ob_.__exit__(None, None, None)
                sb_.__exit__(None, None, None)

    nc.finalize()
    _nc_cache[key] = nc
    return nc


_HEAD_ORDER = [0, 4, 1, 5, 2, 6, 3, 7]


def _perm_wq(wq, g):
    cols = wq[:, 512 * g:512 * (g + 1)].reshape(D, 8, DH)
    return np.ascontiguousarray(cols[:, _HEAD_ORDER].reshape(D, 512)).astype(np.float16)


def _perm_wo(wo, g):
    rows = wo[512 * g:512 * (g + 1), :].reshape(8, DH, D)
    return np.ascontiguousarray(rows[_HEAD_ORDER].reshape(512, D)).astype(np.float16)


def _in_maps(x, wq, wk, wv, wo):
    in_maps = []
    for c in range(NCORES):
        bi, g = c // 4, c % 4
        in_maps.append({
            "xt": np.ascontiguousarray(x[bi].T).astype(np.float16),
            "wq": _perm_wq(wq, g),
            "wk": np.ascontiguousarray(wk[:, 128 * g:128 * (g + 1)]).astype(np.float16),
            "wv": np.ascontiguousarray(wv[:, 128 * g:128 * (g + 1)]).astype(np.float16),
            "wo": _perm_wo(wo, g),
        })
    return in_maps


def kernel(x, wq, wk, wv, wo, attention_mask=None, **_ignored):
    from concourse.bass_utils import run_bass_kernel_spmd

    x = np.asarray(x, dtype=np.float32)
    wq = np.asarray(wq, dtype=np.float32)
    wk = np.asarray(wk, dtype=np.float32)
    wv = np.asarray(wv, dtype=np.float32)
    wo = np.asarray(wo, dtype=np.float32)

    nc = _build()
    res = run_bass_kernel_spmd(nc, _in_maps(x, wq, wk, wv, wo), list(range(NCORES)))
    y = np.zeros((B, T, D), dtype=np.float32)
    for c in range(NCORES):
        y[c // 4] += res.results[c]["out"].astype(np.float32)
    return y
